# revision 11
# baseline (speedup 1.0000x reference)
"""Trainium2 Bass kernel for nn_MultiHeadAttention (B=1, S=4096, d_model=768, 12 heads).

Sharding (8 cores): 2 head-groups (6 heads / 384 channels each) x 4 query-blocks
(1024 rows each).  Each core computes its head-group's contribution to the output
projection for its query block; the host sums the two head-group partials and adds
the constant bias terms.

Device algorithm per core (all layouts chosen so no on-device transposes needed):
  qT [384,1024]  = (Wq_g^T Qb^T) * s + bq_g * s        (s = 1/sqrt(768))
  kT [384,4096]  = Wk_g^T K^T                          (bk drops out of softmax)
  v  [4096,384]  = V Wv_g  (+ ones column per head)    (bv folded into host consts)
  scoresT[j,i]   = sum_d kT[d,j] qT[d,i]               (per head, psum)
  expT           = exp(scoresT)                        (ACT, no max subtraction:
                                                        |scores| <~ 1.5)
  pv[m,i]        = sum_j v_ext[j,m] expT[j,i]          (m<64: out_u^T, m=64: l)
  attn_T         = pv[0:64] * (1/l broadcast)
  OUT [1024,768] = attn_T^T @ Wo_g                     (partial, pre-bias)
Host: out[b] = OUT[g0,b] + OUT[g1,b] + (bv @ Wo + bo); plus exact bias algebra:
  bk contributes a per-row constant to scores -> softmax invariant -> dropped.
"""

import sys

sys.path.insert(0, "/opt/trn_rl_repo")

import numpy as np

import concourse.bass as bass
import concourse.mybir as mybir
from concourse.bass import ts, ds
from concourse.bass_utils import run_bass_kernel_spmd
from concourse.tile import TileContext

D_MODEL = 768
S = 4096
NH = 12
HD = 64
HG = 2                  # head groups
QB = 4                  # query blocks
C = D_MODEL // HG       # 384 channels per group
NHL = NH // HG          # 6 heads per group
QR = S // QB            # 1024 query rows per block
NCORES = 8
SCALE = float(1.0 / np.sqrt(np.float32(D_MODEL)))

F32 = mybir.dt.float32
F32R = mybir.dt.float32r
AF = mybir.ActivationFunctionType


def _r(ap):
    """View an fp32 AP as float32r for single-pass PE matmuls."""
    return ap.bitcast(F32R)


def _split_excess_waits(nc, max_waits=1):
    """walrus rejects instructions carrying more than one semaphore wait
    (setupSyncWait 'Too many sync wait commands').  Hoist excess waits onto
    no-op instructions inserted immediately before, on the same engine."""
    n_split = 0
    for f in nc.m.functions:
        for blk in f.blocks:
            new_insts = []
            for inst in blk.instructions:
                si = inst.sync_info
                if si is not None and si.on_wait and len(si.on_wait) > max_waits:
                    waits = list(si.on_wait)
                    keep = waits[-max_waits:]
                    extra = waits[:-max_waits]
                    for i in range(0, len(extra), max_waits):
                        chunk = extra[i : i + max_waits]
                        nop = mybir.InstNoOp(
                            name=f"{inst.name}_wsplit_{i}",
                            ins=[],
                            outs=[],
                            engine=inst.engine,
                            sync_info=mybir.SyncInfo(on_wait=chunk, on_update=[]),
                        )
                        new_insts.append(nop)
                        n_split += 1
                    si.on_wait = keep
                new_insts.append(inst)
            blk.instructions = new_insts
    return n_split


def _emit_body(nc, tc, io, use_f32r=True):
    QT, KT, VT, WQ, WK, WV, WO, BQ, OUT = (
        io["QT"], io["KT"], io["VT"], io["WQ"], io["WK"], io["WV"], io["WO"],
        io["BQ"], io["OUT"],
    )
    # float32r end-to-end: DRAM inputs are declared f32r, engine-produced
    # matmul operands are written as f32r (DVE/ACT round on write), so the
    # BIR verifier's "rounded to FP32r" rule is satisfied everywhere.
    DT = F32R if use_f32r else F32

    consts = tc.alloc_tile_pool(name="consts", bufs=1)
    big = tc.alloc_tile_pool(name="big", bufs=1)

    # ---- weights -> SBUF ----
    wq_t, wk_t, wv_t = [], [], []
    for i in range(6):
        wq = consts.tile([128, C], DT, tag=f"wq{i}", name=f"wq{i}")
        nc.sync.dma_start(out=wq, in_=WQ[ts(i, 128), :])
        wq_t.append(wq)
        wk = consts.tile([128, C], DT, tag=f"wk{i}", name=f"wk{i}")
        nc.sync.dma_start(out=wk, in_=WK[ts(i, 128), :])
        wk_t.append(wk)
        wv = consts.tile([128, C], DT, tag=f"wv{i}", name=f"wv{i}")
        nc.sync.dma_start(out=wv, in_=WV[ts(i, 128), :])
        wv_t.append(wv)
    wo_t = []
    for p in range(3):
        wo = consts.tile([128, D_MODEL], DT, tag=f"wo{p}", name=f"wo{p}")
        nc.sync.dma_start(out=wo, in_=WO[ts(p, 128), :])
        wo_t.append(wo)
    bq_t = []
    for p in range(3):
        bq = consts.tile([128, 1], F32, tag=f"bq{p}", name=f"bq{p}")
        nc.sync.dma_start(out=bq, in_=BQ[ts(p, 128)].rearrange("(p one) -> p one", one=1))
        bq_t.append(bq)
    ones64 = consts.tile([1, 64], F32, tag="ones64", name="ones64")
    nc.vector.memset(ones64, 1.0)
    # f32 ones source for the v_ext ones column (memset can't write f32r;
    # a DVE copy rounds f32 -> f32r)
    ones_col = consts.tile([128, NHL], F32, tag="ones_col", name="ones_col")
    nc.vector.memset(ones_col, 1.0)

    # ---- persistent activations ----
    qT_t = [big.tile([128, QR], DT, tag=f"qT{p}", name=f"qT{p}") for p in range(3)]
    kT_t = [big.tile([128, S], DT, tag=f"kT{p}", name=f"kT{p}") for p in range(3)]
    vext_t = [
        big.tile([128, NHL, HD + 1], DT, tag=f"vx{j}", name=f"vx{j}")
        for j in range(S // 128)
    ]

    # ================= phase A: projections =================
    instream = tc.alloc_tile_pool(name="instream", bufs=2)
    psA = tc.alloc_tile_pool(name="psA", bufs=4, space="PSUM")

    # qproj: qT = (WQ^T QT) * s + bq*s
    for qc in range(QR // 512):
        qin = [instream.tile([128, 512], DT, tag=f"xin{i}", name=f"qin{i}_{qc}") for i in range(6)]
        for i in range(6):
            nc.sync.dma_start(out=qin[i], in_=QT[ts(i, 128), ts(qc, 512)])
        for p in range(3):
            ps = psA.tile([128, 512], F32, tag="psA", name=f"ps_q{p}_{qc}")
            for i in range(6):
                nc.tensor.matmul(
                    ps, lhsT=wq_t[i][:, ts(p, 128)], rhs=qin[i],
                    start=(i == 0), stop=(i == 5),
                )
            nc.scalar.activation(
                qT_t[p][:, ts(qc, 512)], ps, AF.Identity, bias=bq_t[p], scale=SCALE
            )

    # kproj: kT = WK^T KT
    for ck in range(S // 512):
        kin = [instream.tile([128, 512], DT, tag=f"xin{i}", name=f"kin{i}_{ck}") for i in range(6)]
        for i in range(6):
            nc.sync.dma_start(out=kin[i], in_=KT[ts(i, 128), ts(ck, 512)])
        for p in range(3):
            ps = psA.tile([128, 512], F32, tag="psA", name=f"ps_k{p}_{ck}")
            for i in range(6):
                nc.tensor.matmul(
                    ps, lhsT=wk_t[i][:, ts(p, 128)], rhs=kin[i],
                    start=(i == 0), stop=(i == 5),
                )
            nc.vector.tensor_copy(kT_t[p][:, ts(ck, 512)], ps)

    # vproj: v[key, ch] = sum_in VT[in, key] WV[in, ch], written per-head with a
    # ones column appended (lhsT for the pv matmul).
    for ck in range(S // 512):
        vin = [instream.tile([128, 512], DT, tag=f"xin{i}", name=f"vin{i}_{ck}") for i in range(6)]
        for i in range(6):
            nc.sync.dma_start(out=vin[i], in_=VT[ts(i, 128), ts(ck, 512)])
        for ksub in range(4):
            j = ck * 4 + ksub
            ps = psA.tile([128, C], F32, tag="psA", name=f"ps_v{j}")
            for i in range(6):
                nc.tensor.matmul(
                    ps, lhsT=vin[i][:, ts(ksub, 128)], rhs=wv_t[i],
                    start=(i == 0), stop=(i == 5),
                )
            nc.vector.tensor_copy(
                vext_t[j][:, :, 0:HD], ps.rearrange("p (h d) -> p h d", h=NHL)
            )
            nc.vector.tensor_copy(vext_t[j][:, :, HD], ones_col)

    psA.release()
    instream.release()

    # ================= phase B: attention =================
    expp = tc.alloc_tile_pool(name="expp", bufs=4)
    attnp = tc.alloc_tile_pool(name="attnp", bufs=1)
    outp = tc.alloc_tile_pool(name="outp", bufs=3)
    smallp = tc.alloc_tile_pool(name="smallp", bufs=2)
    psS = tc.alloc_tile_pool(name="psS", bufs=2, space="PSUM")
    psV = tc.alloc_tile_pool(name="psV", bufs=2, space="PSUM")
    psO = tc.alloc_tile_pool(name="psO", bufs=1, space="PSUM")

    NKT = S // 128          # 32 key tiles
    GK = 2                  # key tiles per exp group
    for qc in range(QR // 512):
        attn_tiles = []
        for p in range(3):
            at = attnp.tile([128, 512], DT, tag=f"attn{qc}_{p}", name=f"attn{qc}_{p}")
            attn_tiles.append(at)
            # pv accumulators for the pair's two heads, [65, 512]: rows 0-63
            # out_u^T, row 64 = l (softmax denominator), via the ones column.
            pvh = [
                psV.tile([HD + 1, 512], F32, tag="pv", name=f"pv{qc}_{p}_{h}")
                for h in range(2)
            ]
            for grp in range(NKT // GK):
                es = []
                for h in range(2):
                    sp = psS.tile([128, GK, 512], F32, tag="psS", name=f"sp{qc}_{p}_{grp}_{h}")
                    for kt in range(GK):
                        j = grp * GK + kt
                        nc.tensor.matmul(
                            sp[:, kt, :],
                            lhsT=kT_t[p][ds(64 * h, 64), ts(j, 128)],
                            rhs=qT_t[p][ds(64 * h, 64), ts(qc, 512)],
                            start=True, stop=True,
                        )
                    e = expp.tile([128, GK, 512], DT, tag="exp", name=f"e{qc}_{p}_{grp}_{h}")
                    nc.scalar.activation(e, sp, AF.Exp)
                    es.append(e)
                for h in range(2):
                    for kt in range(GK):
                        j = grp * GK + kt
                        nc.tensor.matmul(
                            pvh[h],
                            lhsT=vext_t[j][:, p * 2 + h, :],
                            rhs=es[h][:, kt, :],
                            start=(j == 0), stop=(j == NKT - 1),
                        )
            # normalize: attn_T rows [64h, 64h+64) = pv[0:64] * (1/l)
            for h in range(2):
                rr = smallp.tile([1, 512], F32, tag=f"rr{h}", name=f"rr{qc}_{p}_{h}")
                nc.vector.reciprocal(rr, pvh[h][ds(HD, 1), :])
                rbc = psV.tile([64, 512], F32, tag="rbc", bufs=1, name=f"rbc{qc}_{p}_{h}")
                # broadcast rr across 64 partitions: ones64^T @ rr  (plain f32:
                # multiply-by-1.0 is exact)
                nc.tensor.matmul(rbc, lhsT=ones64, rhs=rr, start=True, stop=True)
                pv_sb = smallp.tile([64, 512], F32, tag="pv_sb", name=f"pvsb{qc}_{p}_{h}")
                nc.vector.tensor_copy(pv_sb, pvh[h][ds(0, HD), :])
                nc.vector.tensor_mul(at[ds(64 * h, 64), :], pv_sb, rbc)
        # oproj for this q chunk: OUT[qc*512 + qs*128 .. , :] partial
        for qs in range(4):
            ob = outp.tile([128, D_MODEL], F32, tag="ob", name=f"ob{qc}_{qs}")
            for oc in range(2):
                po = psO.tile([128, 384], F32, tag="psO", name=f"po{qc}_{qs}_{oc}")
                for p in range(3):
                    nc.tensor.matmul(
                        po,
                        lhsT=attn_tiles[p][:, ts(qs, 128)],
                        rhs=wo_t[p][:, ts(oc, 384)],
                        start=(p == 0), stop=(p == 2),
                    )
                nc.vector.tensor_copy(ob[:, ts(oc, 384)], po)
            nc.sync.dma_start(out=OUT[ds(qc * 512 + qs * 128, 128), :], in_=ob)

    for pool in [psO, psV, psS, smallp, outp, attnp, expp, big, consts]:
        pool.release()


_nc_cache = {}


def build_nc(reps=1, use_f32r=True, split_waits=True):
    key = (reps, use_f32r, split_waits)
    if key in _nc_cache:
        return _nc_cache[key]
    nc = bass.Bass()
    DT = F32R if use_f32r else F32
    io = {
        "QT": nc.declare_dram_parameter("QT", [D_MODEL, QR], DT, isOutput=False),
        "KT": nc.declare_dram_parameter("KT", [D_MODEL, S], DT, isOutput=False),
        "VT": nc.declare_dram_parameter("VT", [D_MODEL, S], DT, isOutput=False),
        "WQ": nc.declare_dram_parameter("WQ", [D_MODEL, C], DT, isOutput=False),
        "WK": nc.declare_dram_parameter("WK", [D_MODEL, C], DT, isOutput=False),
        "WV": nc.declare_dram_parameter("WV", [D_MODEL, C], DT, isOutput=False),
        "WO": nc.declare_dram_parameter("WO", [C, D_MODEL], DT, isOutput=False),
        "BQ": nc.declare_dram_parameter("BQ", [C], F32, isOutput=False),
        "OUT": nc.declare_dram_parameter("OUT", [QR, D_MODEL], F32, isOutput=True),
    }
    with TileContext(nc) as tc:
        for _ in range(reps):
            _emit_body(nc, tc, io, use_f32r=use_f32r)
    if split_waits:
        _split_excess_waits(nc)
    _nc_cache[key] = nc
    return nc


def make_in_maps(Q, K, V, Wq, bq, Wk, bk, Wv, bv, Wo, bo):
    """Host-side sharding.  Returns (in_maps, host_const) where host_const is
    the [768] vector added to every output row (bv @ Wo + bo)."""
    Qm = np.asarray(Q, np.float32).reshape(S, D_MODEL)
    Km = np.asarray(K, np.float32).reshape(S, D_MODEL)
    Vm = np.asarray(V, np.float32).reshape(S, D_MODEL)
    QT = np.ascontiguousarray(Qm.T)
    KT = np.ascontiguousarray(Km.T)
    VT = np.ascontiguousarray(Vm.T)
    Wq = np.asarray(Wq, np.float32); Wk = np.asarray(Wk, np.float32)
    Wv = np.asarray(Wv, np.float32); Wo = np.asarray(Wo, np.float32)
    bq = np.asarray(bq, np.float32); bv = np.asarray(bv, np.float32)
    bo = np.asarray(bo, np.float32)

    in_maps = []
    for c in range(NCORES):
        g, b = divmod(c, QB)
        ch = slice(g * C, (g + 1) * C)
        in_maps.append({
            "QT": np.ascontiguousarray(QT[:, b * QR : (b + 1) * QR]),
            "KT": KT,
            "VT": VT,
            "WQ": np.ascontiguousarray(Wq[:, ch]),
            "WK": np.ascontiguousarray(Wk[:, ch]),
            "WV": np.ascontiguousarray(Wv[:, ch]),
            "WO": np.ascontiguousarray(Wo[ch, :]),
            # device computes qT = psum*SCALE + BQ, so prescale the bias here
            "BQ": np.ascontiguousarray(bq[ch] * np.float32(SCALE)),
        })
    host_const = (bv @ Wo + bo).astype(np.float32)
    return in_maps, host_const


def kernel(Q, K, V, Wq, bq, Wk, bk, Wv, bv, Wo, bo):
    nc = build_nc()
    in_maps, host_const = make_in_maps(Q, K, V, Wq, bq, Wk, bk, Wv, bv, Wo, bo)
    res = run_bass_kernel_spmd(nc, in_maps, core_ids=list(range(NCORES)))
    out = np.zeros((S, D_MODEL), np.float32)
    for c in range(NCORES):
        g, b = divmod(c, QB)
        out[b * QR : (b + 1) * QR, :] += res.results[c]["OUT"]
    out += host_const[None, :]
    return out.reshape(1, S, D_MODEL)


# revision 20
# speedup vs baseline: 11790.5802x; 11790.5802x over previous
"""Trainium2 Bass kernel for nn_MultiHeadAttention (B=1, S=4096, d_model=768, 12 heads).

Sharding (8 cores): 2 head-groups (6 heads / 384 channels each) x 4 query-blocks
(1024 rows each).  Each core computes its head-group's contribution to the output
projection for its query block; the host sums the two head-group partials and adds
the constant bias terms.

Device algorithm per core (all layouts chosen so no on-device transposes needed):
  qT [384,1024]  = (Wq_g^T Qb^T) * s + bq_g * s        (s = 1/sqrt(768))
  kT [384,4096]  = Wk_g^T K^T                          (bk drops out of softmax)
  v  [4096,384]  = V Wv_g  (+ ones column per head)    (bv folded into host consts)
  scoresT[j,i]   = sum_d kT[d,j] qT[d,i]               (per head, psum)
  expT           = exp(scoresT)                        (ACT, no max subtraction:
                                                        |scores| <~ 1.5)
  pv[m,i]        = sum_j v_ext[j,m] expT[j,i]          (m<64: out_u^T, m=64: l)
  attn_T         = pv[0:64] * (1/l broadcast)
  OUT [1024,768] = attn_T^T @ Wo_g                     (partial, pre-bias)
Host: out[b] = OUT[g0,b] + OUT[g1,b] + (bv @ Wo + bo); plus exact bias algebra:
  bk contributes a per-row constant to scores -> softmax invariant -> dropped.
"""

import sys

sys.path.insert(0, "/opt/trn_rl_repo")

import numpy as np

import concourse.bass as bass
import concourse.mybir as mybir
from concourse.bass import ts, ds
from concourse.bass_utils import run_bass_kernel_spmd
from concourse.tile import TileContext

D_MODEL = 768
S = 4096
NH = 12
HD = 64
HG = 2                  # head groups
QB = 4                  # query blocks
C = D_MODEL // HG       # 384 channels per group
NHL = NH // HG          # 6 heads per group
QR = S // QB            # 1024 query rows per block
NCORES = 8
SCALE = float(1.0 / np.sqrt(np.float32(D_MODEL)))

F32 = mybir.dt.float32
F32R = mybir.dt.float32r
BF16 = mybir.dt.bfloat16
AF = mybir.ActivationFunctionType


def _r(ap):
    """View an fp32 AP as float32r for single-pass PE matmuls."""
    return ap.bitcast(F32R)


def _split_excess_waits(nc, max_waits=1):
    """walrus rejects instructions carrying more than one semaphore wait
    (setupSyncWait 'Too many sync wait commands').  Hoist excess waits onto
    no-op instructions inserted immediately before, on the same engine."""
    n_split = 0
    for f in nc.m.functions:
        for blk in f.blocks:
            new_insts = []
            for inst in blk.instructions:
                si = inst.sync_info
                if si is not None and si.on_wait and len(si.on_wait) > max_waits:
                    waits = list(si.on_wait)
                    keep = waits[-max_waits:]
                    extra = waits[:-max_waits]
                    for i in range(0, len(extra), max_waits):
                        chunk = extra[i : i + max_waits]
                        nop = mybir.InstNoOp(
                            name=f"{inst.name}_wsplit_{i}",
                            ins=[],
                            outs=[],
                            engine=inst.engine,
                            sync_info=mybir.SyncInfo(on_wait=chunk, on_update=[]),
                        )
                        new_insts.append(nop)
                        n_split += 1
                    si.on_wait = keep
                new_insts.append(inst)
            blk.instructions = new_insts
    return n_split


def _emit_body(nc, tc, io, use_f32r=True, stages=("proj", "attn", "oproj"), att_bf16=False,
               in_bf16=False, prof=None):
    QT, KT, VT, WQ, WK, WV, WO, BQ, OUT = (
        io["QT"], io["KT"], io["VT"], io["WQ"], io["WK"], io["WV"], io["WO"],
        io["BQ"], io["OUT"],
    )
    # float32r end-to-end: DRAM inputs are declared f32r, engine-produced
    # matmul operands are written as f32r (DVE/ACT round on write), so the
    # BIR verifier's "rounded to FP32r" rule is satisfied everywhere.
    DT = F32R if use_f32r else F32
    # attention-side dtype: bf16 halves nothing in cycle count but avoids the
    # slow f32r self-loading weight path and enables FWL on the PE
    DA = BF16 if att_bf16 else DT
    # input/projection-side dtype: bf16 halves the dominant input DMA traffic
    DI = BF16 if in_bf16 else DT

    consts = tc.alloc_tile_pool(name="consts", bufs=1)
    big = tc.alloc_tile_pool(name="big", bufs=1)

    # ---- weights -> SBUF ----
    wq_t, wk_t, wv_t = [], [], []
    for i in range(6):
        wq = consts.tile([128, C], DI, tag=f"wq{i}", name=f"wq{i}")
        d0 = nc.sync.dma_start(out=wq, in_=WQ[ts(i, 128), :])
        if prof is not None and i == 0:
            prof.snap(0, d0)
        wq_t.append(wq)
        wk = consts.tile([128, C], DI, tag=f"wk{i}", name=f"wk{i}")
        nc.sync.dma_start(out=wk, in_=WK[ts(i, 128), :])
        wk_t.append(wk)
        wv = consts.tile([128, C], DI, tag=f"wv{i}", name=f"wv{i}")
        nc.sync.dma_start(out=wv, in_=WV[ts(i, 128), :])
        wv_t.append(wv)
    wo_t = []
    for p in range(3):
        wo = consts.tile([128, D_MODEL], DT, tag=f"wo{p}", name=f"wo{p}")
        nc.sync.dma_start(out=wo, in_=WO[ts(p, 128), :])
        wo_t.append(wo)
    bq_t = []
    for p in range(3):
        bq = consts.tile([128, 1], F32, tag=f"bq{p}", name=f"bq{p}")
        nc.sync.dma_start(out=bq, in_=BQ[ts(p, 128)].rearrange("(p one) -> p one", one=1))
        bq_t.append(bq)
    ones64 = consts.tile([1, 64], F32, tag="ones64", name="ones64")
    nc.vector.memset(ones64, 1.0)
    # f32 ones source for the v_ext ones column (memset can't write f32r;
    # a DVE copy rounds f32 -> f32r)
    ones_col = consts.tile([128, NHL], F32, tag="ones_col", name="ones_col")
    nc.vector.memset(ones_col, 1.0)

    # ---- persistent activations ----
    qT_t = [big.tile([128, QR], DA, tag=f"qT{p}", name=f"qT{p}") for p in range(3)]
    kT_t = [big.tile([128, S], DA, tag=f"kT{p}", name=f"kT{p}") for p in range(3)]
    vext_t = [
        big.tile([128, NHL, HD + 1], DA, tag=f"vx{j}", name=f"vx{j}")
        for j in range(S // 128)
    ]

    # ================= phase A: projections =================
    instream = tc.alloc_tile_pool(name="instream", bufs=2)
    psA = tc.alloc_tile_pool(name="psA", bufs=4, space="PSUM")

    # qproj: qT = (WQ^T QT) * s + bq*s
    for qc in range(QR // 512):
        qin = [instream.tile([128, 512], DI, tag=f"xin{i}", name=f"qin{i}_{qc}") for i in range(6)]
        for i in range(6):
            nc.sync.dma_start(out=qin[i], in_=QT[ts(i, 128), ts(qc, 512)])
        for p in range(3):
            ps = psA.tile([128, 512], F32, tag="psA", name=f"ps_q{p}_{qc}")
            for i in range(6):
                nc.tensor.matmul(
                    ps, lhsT=wq_t[i][:, ts(p, 128)], rhs=qin[i],
                    start=(i == 0), stop=(i == 5),
                )
            gate = nc.scalar.activation(
                qT_t[p][:, ts(qc, 512)], ps, AF.Identity, bias=bq_t[p], scale=SCALE
            )
            if prof is not None and qc == QR // 512 - 1 and p == 2:
                prof.snap(1, gate)

    # kproj: kT = WK^T KT
    for ck in range(S // 512):
        kin = [instream.tile([128, 512], DI, tag=f"xin{i}", name=f"kin{i}_{ck}") for i in range(6)]
        for i in range(6):
            nc.sync.dma_start(out=kin[i], in_=KT[ts(i, 128), ts(ck, 512)])
        for p in range(3):
            ps = psA.tile([128, 512], F32, tag="psA", name=f"ps_k{p}_{ck}")
            for i in range(6):
                nc.tensor.matmul(
                    ps, lhsT=wk_t[i][:, ts(p, 128)], rhs=kin[i],
                    start=(i == 0), stop=(i == 5),
                )
            gate = nc.vector.tensor_copy(kT_t[p][:, ts(ck, 512)], ps)
            if prof is not None and ck == S // 512 - 1 and p == 2:
                prof.snap(2, gate)

    # vproj: v[key, ch] = sum_in VT[in, key] WV[in, ch], written per-head with a
    # ones column appended (lhsT for the pv matmul).
    for ck in range(S // 512):
        vin = [instream.tile([128, 512], DI, tag=f"xin{i}", name=f"vin{i}_{ck}") for i in range(6)]
        for i in range(6):
            nc.sync.dma_start(out=vin[i], in_=VT[ts(i, 128), ts(ck, 512)])
        for ksub in range(4):
            j = ck * 4 + ksub
            ps = psA.tile([128, C], F32, tag="psA", name=f"ps_v{j}")
            for i in range(6):
                nc.tensor.matmul(
                    ps, lhsT=vin[i][:, ts(ksub, 128)], rhs=wv_t[i],
                    start=(i == 0), stop=(i == 5),
                )
            nc.vector.tensor_copy(
                vext_t[j][:, :, 0:HD], ps.rearrange("p (h d) -> p h d", h=NHL)
            )
            gate = nc.vector.tensor_copy(vext_t[j][:, :, HD], ones_col)
            if prof is not None and j == S // 128 - 1:
                prof.snap(3, gate)

    psA.release()
    instream.release()

    # ================= phase B: attention =================
    expp = tc.alloc_tile_pool(name="expp", bufs=4)
    attnp = tc.alloc_tile_pool(name="attnp", bufs=1)
    outp = tc.alloc_tile_pool(name="outp", bufs=3)
    smallp = tc.alloc_tile_pool(name="smallp", bufs=2)
    psS = tc.alloc_tile_pool(name="psS", bufs=2, space="PSUM")
    psV = tc.alloc_tile_pool(name="psV", bufs=2, space="PSUM")
    psO = tc.alloc_tile_pool(name="psO", bufs=1, space="PSUM")

    do_attn = "attn" in stages
    do_oproj = "oproj" in stages
    NKT = S // 128          # 32 key tiles
    GK = 2                  # key tiles per exp group
    for qc in range(QR // 512):
        attn_tiles = []
        for p in range(3):
            if not do_attn:
                break
            at = attnp.tile([128, 512], DT, tag=f"attn{qc}_{p}", name=f"attn{qc}_{p}")
            attn_tiles.append(at)
            # pv accumulators for the pair's two heads, [65, 512]: rows 0-63
            # out_u^T, row 64 = l (softmax denominator), via the ones column.
            pvh = [
                psV.tile([HD + 1, 512], F32, tag="pv", name=f"pv{qc}_{p}_{h}")
                for h in range(2)
            ]
            for grp in range(NKT // GK):
                es = []
                for h in range(2):
                    sp = psS.tile([128, GK, 512], F32, tag="psS", name=f"sp{qc}_{p}_{grp}_{h}")
                    for kt in range(GK):
                        j = grp * GK + kt
                        nc.tensor.matmul(
                            sp[:, kt, :],
                            lhsT=kT_t[p][ds(64 * h, 64), ts(j, 128)],
                            rhs=qT_t[p][ds(64 * h, 64), ts(qc, 512)],
                            start=True, stop=True,
                        )
                    e = expp.tile([128, GK, 512], DA, tag="exp", name=f"e{qc}_{p}_{grp}_{h}")
                    offload = (h == 1)
                    if not offload:
                        # ACT reads PSUM at ~2.3 cyc/elem (vs 1.17 from SBUF);
                        # split the softmax between ACT-direct and a DVE
                        # evacuation + ACT-from-SBUF to balance the engines.
                        nc.scalar.activation(e, sp, AF.Exp)
                    else:
                        s_sb = expp.tile([128, GK, 512], F32, tag="s_sb",
                                         name=f"ssb{qc}_{p}_{grp}_{h}", bufs=3)
                        nc.vector.tensor_copy(s_sb, sp)
                        nc.scalar.activation(e, s_sb, AF.Exp)
                    es.append(e)
                for h in range(2):
                    for kt in range(GK):
                        j = grp * GK + kt
                        nc.tensor.matmul(
                            pvh[h],
                            lhsT=vext_t[j][:, p * 2 + h, :],
                            rhs=es[h][:, kt, :],
                            start=(j == 0), stop=(j == NKT - 1),
                        )
            # normalize: attn_T rows [64h, 64h+64) = pv[0:64] * (1/l)
            for h in range(2):
                rr = smallp.tile([1, 512], F32, tag=f"rr{h}", name=f"rr{qc}_{p}_{h}")
                nc.vector.reciprocal(rr, pvh[h][ds(HD, 1), :])
                rbc = psV.tile([64, 512], F32, tag="rbc", bufs=1, name=f"rbc{qc}_{p}_{h}")
                # broadcast rr across 64 partitions: ones64^T @ rr  (plain f32:
                # multiply-by-1.0 is exact)
                nc.tensor.matmul(rbc, lhsT=ones64, rhs=rr, start=True, stop=True)
                pv_sb = smallp.tile([64, 512], F32, tag="pv_sb", name=f"pvsb{qc}_{p}_{h}")
                nc.vector.tensor_copy(pv_sb, pvh[h][ds(0, HD), :])
                gate = nc.vector.tensor_mul(at[ds(64 * h, 64), :], pv_sb, rbc)
                if prof is not None and h == 1:
                    prof.snap(4 + qc * 3 + p, gate)
        # oproj for this q chunk: OUT[qc*512 + qs*128 .. , :] partial
        for qs in range(4):
            if not (do_attn and do_oproj):
                break
            ob = outp.tile([128, D_MODEL], F32, tag="ob", name=f"ob{qc}_{qs}")
            for oc in range(2):
                po = psO.tile([128, 384], F32, tag="psO", name=f"po{qc}_{qs}_{oc}")
                for p in range(3):
                    nc.tensor.matmul(
                        po,
                        lhsT=attn_tiles[p][:, ts(qs, 128)],
                        rhs=wo_t[p][:, ts(oc, 384)],
                        start=(p == 0), stop=(p == 2),
                    )
                nc.vector.tensor_copy(ob[:, ts(oc, 384)], po)
            gate = nc.sync.dma_start(out=OUT[ds(qc * 512 + qs * 128, 128), :], in_=ob)
            if prof is not None and qs == 3:
                prof.snap(10 + qc, gate)

    for pool in [psO, psV, psS, smallp, outp, attnp, expp, big, consts]:
        pool.release()


_nc_cache = {}


PROF_LK = 256           # ladder length (ticks)
PROF_TICK_CYC = 4800    # NX cycles per tick  (~4us at 1.2 GHz)
PROF_NSNAP = 12


class _Prof:
    """On-device sampling profiler: a GPSIMD tick ladder (sequencer-only
    stores + fixed-cycle nops, invisible to Tile's dep tracker) plus snapshot
    DMAs of the tick buffer gated on phase-completion instructions."""

    def __init__(self, nc, prog_ap, PROG):
        self.nc = nc
        self.prog_ap = prog_ap
        self.PROG = PROG

    def snap(self, idx, gate):
        from concourse.tile_rust import add_dep_helper
        d = self.nc.sync.dma_start(out=self.PROG[ds(idx, 1), :], in_=self.prog_ap)
        add_dep_helper(d.ins, gate.ins, sync=True, reason=f"prof snap {idx}")


def _emit_prof_ladder(nc, prog_ap):
    """Emit (post-Tile) the Pool tick ladder, then relocate it to just after
    Pool's preamble-barrier instructions so it runs concurrently with the
    kernel body."""
    ladder = []
    reg_ctx = nc.gpsimd.register("prof_tick")
    reg = reg_ctx.__enter__()
    z = nc.gpsimd.reg_alu(reg, 0, 0, mybir.AluOpType.add)
    ladder.append(z.ins)
    for i in range(PROF_LK):
        s = nc.gpsimd.store(prog_ap[0:1, ds(i, 1)], reg)
        ladder.append(s.ins)
    for i in range(PROF_LK):
        a = nc.gpsimd.reg_alu(reg, reg, 1, mybir.AluOpType.add)
        ladder.append(a.ins)
        s = nc.gpsimd.store(prog_ap[0:1, ds(i, 1)], reg)
        ladder.append(s.ins)
        n = nc.gpsimd.nop(cycle_cnt=PROF_TICK_CYC, nofuse=True)
        ladder.append(n.ins)
    ladder_set = set(id(x) for x in ladder)
    f = nc.m.functions[0]
    # remove from wherever they were appended
    for blk in f.blocks:
        blk.instructions = [x for x in blk.instructions if id(x) not in ladder_set]
    # insert at the start of the TileContext body block so Pool runs the
    # ladder concurrently with the kernel (Pool is otherwise unused there)
    for blk in f.blocks:
        if blk.name.startswith("tile_context"):
            blk.instructions[0:0] = ladder
            return
    raise RuntimeError("profiler: no tile_context block found for tick ladder")


def build_nc(reps=1, use_f32r=True, split_waits=True, stages=("proj", "attn", "oproj"),
             timing_mode=False, att_bf16=False, in_bf16=False, profile_ladder=False):
    key = (reps, use_f32r, split_waits, tuple(stages), timing_mode, att_bf16, in_bf16,
           profile_ladder)
    if key in _nc_cache:
        return _nc_cache[key]
    nc = bass.Bass()
    DT = F32R if use_f32r else F32
    # attention-side dtype: bf16 halves nothing in cycle count but avoids the
    # slow f32r self-loading weight path and enables FWL on the PE
    DA = BF16 if att_bf16 else DT
    # input/projection-side dtype: bf16 halves the dominant input DMA traffic
    DI = BF16 if in_bf16 else DT
    if timing_mode:
        # timing-only variant: big tensors live in Internal DRAM so per-call
        # host->device staging is negligible; numerics are garbage.
        nc.declare_dram_parameter("DUMMY", [1, 128], F32, isOutput=False)
        io = {
            "QT": nc.dram_tensor("QT", [D_MODEL, QR], DI),
            "KT": nc.dram_tensor("KT", [D_MODEL, S], DI),
            "VT": nc.dram_tensor("VT", [D_MODEL, S], DI),
            "WQ": nc.dram_tensor("WQ", [D_MODEL, C], DI),
            "WK": nc.dram_tensor("WK", [D_MODEL, C], DI),
            "WV": nc.dram_tensor("WV", [D_MODEL, C], DI),
            "WO": nc.dram_tensor("WO", [C, D_MODEL], DT),
            "BQ": nc.dram_tensor("BQ", [C], F32),
            "OUT": nc.declare_dram_parameter("OUT", [QR, D_MODEL], F32, isOutput=True),
        }
    else:
        io = {
            "QT": nc.declare_dram_parameter("QT", [D_MODEL, QR], DI, isOutput=False),
            "KT": nc.declare_dram_parameter("KT", [D_MODEL, S], DI, isOutput=False),
            "VT": nc.declare_dram_parameter("VT", [D_MODEL, S], DI, isOutput=False),
            "WQ": nc.declare_dram_parameter("WQ", [D_MODEL, C], DI, isOutput=False),
            "WK": nc.declare_dram_parameter("WK", [D_MODEL, C], DI, isOutput=False),
            "WV": nc.declare_dram_parameter("WV", [D_MODEL, C], DI, isOutput=False),
            "WO": nc.declare_dram_parameter("WO", [C, D_MODEL], DT, isOutput=False),
            "BQ": nc.declare_dram_parameter("BQ", [C], F32, isOutput=False),
            "OUT": nc.declare_dram_parameter("OUT", [QR, D_MODEL], F32, isOutput=True),
        }
    prof = None
    prog_ap = None
    if profile_ladder:
        PROG = nc.declare_dram_parameter(
            "PROG", [PROF_NSNAP, PROF_LK], mybir.dt.int32, isOutput=True)
        prog_ap = nc.alloc_sbuf_tensor("prog_ticks", [1, PROF_LK], mybir.dt.int32).ap()
        prof = _Prof(nc, prog_ap, PROG)
    with TileContext(nc) as tc:
        for _ in range(reps):
            _emit_body(nc, tc, io, use_f32r=use_f32r, stages=stages, att_bf16=att_bf16,
                       in_bf16=in_bf16, prof=prof)
    if profile_ladder:
        _emit_prof_ladder(nc, prog_ap)
    if split_waits:
        _split_excess_waits(nc)
    _nc_cache[key] = nc
    return nc


def make_in_maps(Q, K, V, Wq, bq, Wk, bk, Wv, bv, Wo, bo, in_bf16=False):
    """Host-side sharding.  Returns (in_maps, host_const) where host_const is
    the [768] vector added to every output row (bv @ Wo + bo)."""
    Qm = np.asarray(Q, np.float32).reshape(S, D_MODEL)
    Km = np.asarray(K, np.float32).reshape(S, D_MODEL)
    Vm = np.asarray(V, np.float32).reshape(S, D_MODEL)
    QT = np.ascontiguousarray(Qm.T)
    KT = np.ascontiguousarray(Km.T)
    VT = np.ascontiguousarray(Vm.T)
    Wq = np.asarray(Wq, np.float32); Wk = np.asarray(Wk, np.float32)
    Wv = np.asarray(Wv, np.float32); Wo = np.asarray(Wo, np.float32)
    bq = np.asarray(bq, np.float32); bv = np.asarray(bv, np.float32)
    bo = np.asarray(bo, np.float32)

    import ml_dtypes
    def cvt(a):
        return np.ascontiguousarray(a).astype(ml_dtypes.bfloat16) if in_bf16 \
            else np.ascontiguousarray(a)
    in_maps = []
    for c in range(NCORES):
        g, b = divmod(c, QB)
        ch = slice(g * C, (g + 1) * C)
        in_maps.append({
            "QT": cvt(QT[:, b * QR : (b + 1) * QR]),
            "KT": cvt(KT),
            "VT": cvt(VT),
            "WQ": cvt(Wq[:, ch]),
            "WK": cvt(Wk[:, ch]),
            "WV": cvt(Wv[:, ch]),
            "WO": np.ascontiguousarray(Wo[ch, :]),
            # device computes qT = psum*SCALE + BQ, so prescale the bias here
            "BQ": np.ascontiguousarray(bq[ch] * np.float32(SCALE)),
        })
    host_const = (bv @ Wo + bo).astype(np.float32)
    return in_maps, host_const


def kernel(Q, K, V, Wq, bq, Wk, bk, Wv, bv, Wo, bo):
    nc = build_nc()
    in_maps, host_const = make_in_maps(Q, K, V, Wq, bq, Wk, bk, Wv, bv, Wo, bo)
    res = run_bass_kernel_spmd(nc, in_maps, core_ids=list(range(NCORES)))
    out = np.zeros((S, D_MODEL), np.float32)
    for c in range(NCORES):
        g, b = divmod(c, QB)
        out[b * QR : (b + 1) * QR, :] += res.results[c]["OUT"]
    out += host_const[None, :]
    return out.reshape(1, S, D_MODEL)
